# revision 17
# baseline (speedup 1.0000x reference)
"""MobileViTV2 block kernel for 8 TRN2 NeuronCores (data-parallel over batch).

Layout: d-major everywhere — features on SBUF partitions, tokens on the free
axis, token order n = t*1024 + h*32 + w (natural). Patch id of a token is
(h&1, w&1), recoverable from free-index bits, so attention runs on natural
order with strided sub-APs and nothing is ever transposed or scattered.

Per core (one batch element):
  P1: dwconv3x3x3 as 27 diagonal matmuls on the PE (diag weight matrices are
      prebuilt host-side) + SiLU + pw1 matmul -> z0 chunks; fused block-0
      "pass A": LN1 stats via ones-matmul, rsqrt as exp(-0.5*ln(var+eps))
      (stays in the natural_log_exp ACT table set together with softmax's
      exp), normalize, q+v matmul (the k GEMM is algebraically removed:
      cv = Wk^T (sum_n cs_n * zn_n), so we only accumulate s = sum cs*zn
      into pinned PSUM banks via identity matmuls and apply Wk once per
      block at finalize), exp(q) (no max-sub; LN-bounded), v=relu(v+b)
      -> DRAM.
  P2: block-0 "pass B" (v*cv -> wo -> +z residual, LN2+FFN residual) fused
      with block-1 pass A.  v*cv runs on DVE tensor_scalar (per-partition
      scalars), not GPSIMD.
  P3: block-1 pass B fused with pw2 -> out.
GEMMs run as float32r (full-rate fp32). All host-side prep (LN-gain folding
into the next GEMM's weights, qkv split, conv diag matrices) happens in
numpy inside kernel().
"""

import sys

sys.path.insert(0, "/opt/trn_rl_repo")
import os
import numpy as np
from contextlib import ExitStack

import concourse.bass as bass
import concourse.mybir as mybir
import concourse.tile as tile
from concourse import bacc
from concourse.bass_utils import run_bass_kernel_spmd

F32 = mybir.dt.float32
F32R = mybir.dt.float32r
AF = mybir.ActivationFunctionType
OP = mybir.AluOpType

B, C, T, H, W = 8, 256, 16, 32, 32
D, OUTC, NBLK, FF = 384, 256, 2, 768
NTOK = T * H * W  # 16384 tokens per batch element
CH = 512  # tokens per chunk
NCH = NTOK // CH  # 32
PW = 34  # padded spatial row
PSL = PW * PW  # padded slice 1156
EPS = 1e-5
NDIAG = 2 * 27  # conv diag matrices (ctile-major, tap-minor)

# build stage knob for incremental bring-up (3 = full kernel)
STAGE = int(os.environ.get("KERNEL_STAGE", "3"))
# CoreSim doesn't implement Silu; swap to sim-supported funcs when
# hunting memory/sync bugs (numerics intentionally wrong then).
SIM_SAFE = bool(int(os.environ.get("KERNEL_SIM_SAFE", "0")))
SKIP = set(os.environ.get("KERNEL_SKIP", "").split(","))


def _w_tiles(nc, wpool, name, dram, kdim, mdim, as_f32r=True):
    """Load a [K, M] DRAM weight as ceil(K/128) SBUF lhsT tiles."""
    tiles = []
    for ki in range((kdim + 127) // 128):
        kk = min(128, kdim - ki * 128)
        t = wpool.tile([128, mdim], F32, tag=f"{name}{ki}")
        dst = t[:kk, :]
        if as_f32r:
            dst = dst.bitcast(F32R)
        nc.sync.dma_start(out=dst, in_=dram[ki * 128 : ki * 128 + kk, :].bitcast(F32R) if as_f32r else dram[ki * 128 : ki * 128 + kk, :])
        tiles.append(t)
    return tiles


def _bias_tile(nc, wpool, name, dram, n):
    """Load a [n,1] DRAM bias as a [128, ceil(n/128)] SBUF tile (col per ktile)."""
    nt = (n + 127) // 128
    t = wpool.tile([128, nt], F32, tag=name)
    for ki in range(nt):
        kk = min(128, n - ki * 128)
        nc.sync.dma_start(out=t[:kk, ki : ki + 1], in_=dram[ki * 128 : ki * 128 + kk, :])
    return t


def patch_view(ap):
    """[p, 512] -> [p, 8, 2, 16, 2]; dims 2/4 are the (ph, pw) patch bits."""
    return ap.rearrange("p (a b c d) -> p a b c d", a=8, b=2, c=16, d=2)


def build():
    nc = bacc.Bacc("TRN2", target_bir_lowering=False, debug=False, num_devices=8)

    x_in = nc.dram_tensor("x", [C, T, H, W], F32, kind="ExternalInput").ap()
    zpad = nc.dram_tensor("zpad", [128, 2 * PSL], F32, kind="ExternalInput").ap()
    dwDiag = nc.dram_tensor("dwDiag", [(NDIAG + 1) * 128, 128], F32, kind="ExternalInput").ap()
    dwB = nc.dram_tensor("dwB", [C, 1], F32, kind="ExternalInput").ap()
    pw1W = nc.dram_tensor("pw1W", [C, D], F32, kind="ExternalInput").ap()
    pw1B = nc.dram_tensor("pw1B", [D, 1], F32, kind="ExternalInput").ap()
    pw2W = nc.dram_tensor("pw2W", [D, OUTC], F32, kind="ExternalInput").ap()
    pw2B = nc.dram_tensor("pw2B", [OUTC, 1], F32, kind="ExternalInput").ap()
    blk = []
    for i in range(NBLK):
        blk.append(
            dict(
                qkvW=nc.dram_tensor(f"qkvW{i}", [D, 1 + 2 * D], F32, kind="ExternalInput").ap(),
                qB=nc.dram_tensor(f"qB{i}", [1, 1], F32, kind="ExternalInput").ap(),
                kB=nc.dram_tensor(f"kB{i}", [D, 1], F32, kind="ExternalInput").ap(),
                vB=nc.dram_tensor(f"vB{i}", [D, 1], F32, kind="ExternalInput").ap(),
                woW=nc.dram_tensor(f"woW{i}", [D, D], F32, kind="ExternalInput").ap(),
                woB=nc.dram_tensor(f"woB{i}", [D, 1], F32, kind="ExternalInput").ap(),
                ff1W=nc.dram_tensor(f"ff1W{i}", [D, FF], F32, kind="ExternalInput").ap(),
                ff1B=nc.dram_tensor(f"ff1B{i}", [FF, 1], F32, kind="ExternalInput").ap(),
                ff2W=nc.dram_tensor(f"ff2W{i}", [FF, D], F32, kind="ExternalInput").ap(),
                ff2B=nc.dram_tensor(f"ff2B{i}", [D, 1], F32, kind="ExternalInput").ap(),
            )
        )
    out = nc.dram_tensor("out", [OUTC, NTOK], F32, kind="ExternalOutput").ap()
    z0 = nc.dram_tensor("z0", [D, NTOK], F32, kind="ExternalOutput").ap()
    z1 = nc.dram_tensor("z1", [D, NTOK], F32, kind="ExternalOutput").ap()
    v0 = nc.dram_tensor("v0", [D, NTOK], F32).ap()
    v1 = nc.dram_tensor("v1", [D, NTOK], F32).ap()
    zsd = nc.dram_tensor("zsd", [NBLK, 4], F32).ap()

    with ExitStack() as ctx:
        tc = ctx.enter_context(tile.TileContext(nc))
        wpool = ctx.enter_context(tc.tile_pool(name="w", bufs=1))
        sp = ctx.enter_context(tc.tile_pool(name="s", bufs=2))
        pp = ctx.enter_context(tc.tile_pool(name="ps", bufs=4, space="PSUM"))
        cvp = ctx.enter_context(tc.tile_pool(name="cv", bufs=1))

        # ---- weights ----
        diag_t = _w_tiles(nc, wpool, "dwDiag", dwDiag, (NDIAG + 1) * 128, 128)
        ident_t = diag_t[NDIAG]  # [128,128] identity (for token-sum accumulation)
        dwb_t = _bias_tile(nc, wpool, "dwB", dwB, C)
        pw1_t = _w_tiles(nc, wpool, "pw1W", pw1W, C, D)
        pw1b_t = _bias_tile(nc, wpool, "pw1B", pw1B, D)
        pw2_t = _w_tiles(nc, wpool, "pw2W", pw2W, D, OUTC)
        pw2b_t = _bias_tile(nc, wpool, "pw2B", pw2B, OUTC)
        bw = []
        for i in range(NBLK):
            bw.append(
                dict(
                    qkv=_w_tiles(nc, wpool, f"qkvW{i}_", blk[i]["qkvW"], D, 1 + 2 * D),
                    qB=_bias_tile(nc, wpool, f"qB{i}", blk[i]["qB"], 1),
                    kB=_bias_tile(nc, wpool, f"kB{i}", blk[i]["kB"], D),
                    vB=_bias_tile(nc, wpool, f"vB{i}", blk[i]["vB"], D),
                    wo=_w_tiles(nc, wpool, f"woW{i}_", blk[i]["woW"], D, D),
                    woB=_bias_tile(nc, wpool, f"woB{i}", blk[i]["woB"], D),
                    ff1=_w_tiles(nc, wpool, f"ff1W{i}_", blk[i]["ff1W"], D, FF),
                    ff1B=_bias_tile(nc, wpool, f"ff1B{i}", blk[i]["ff1B"], FF),
                    ff2=_w_tiles(nc, wpool, f"ff2W{i}_", blk[i]["ff2W"], FF, D),
                    ff2B=_bias_tile(nc, wpool, f"ff2B{i}", blk[i]["ff2B"], D),
                )
            )
        ones_f = wpool.tile([128, 128], F32, tag="onesf")
        nc.vector.memset(ones_f[:], 1.0)
        ones_t = wpool.tile([128, 128], F32, tag="ones")
        nc.scalar.copy(ones_t[:].bitcast(F32R), ones_f[:])
        eps_t = wpool.tile([128, 1], F32, tag="eps")
        nc.vector.memset(eps_t[:], EPS)

        # per-block attention state: Z partials + final cv
        att = []
        for i in range(NBLK):
            att.append(
                dict(
                    Zp=cvp.tile([1, 4, NCH], F32, tag=f"Zp{i}", name=f"Zp{i}"),
                    cvf=cvp.tile([128, 3, 4], F32, tag=f"cvf{i}", name=f"cvf{i}"),  # [p, dtile, patch]
                )
            )
            nc.vector.memset(att[i]["Zp"][:], 1.0)

        def ln_then_qkv_attn(bi, zt, chunk, sacc):
            """Pass A for block bi on an SBUF z chunk [128, 3*512] (f32r-written).

            LN1 stats -> normalize in place -> q+v matmul -> exp(q)/Z partial,
            s += cs*zn via identity-matmul into pinned PSUM, v -> relu -> DRAM.
            """
            a = att[bi]
            wts = bw[bi]
            vdst = v0 if bi == 0 else v1
            # sums via ones-matmul (replicated over partitions)
            ps_s = pp.tile([128, CH], F32, tag="ps")
            ps_q = pp.tile([128, CH], F32, tag="ps")
            for d in range(3):
                sq = sp.tile([128, CH], F32, tag="sq", name="sq")
                nc.scalar.activation(out=sq[:].bitcast(F32R), in_=zt[:, d * CH : (d + 1) * CH], func=AF.Square)
                nc.tensor.matmul(ps_s[:], ones_t[:].bitcast(F32R), zt[:, d * CH : (d + 1) * CH].bitcast(F32R), start=(d == 0), stop=(d == 2))
                nc.tensor.matmul(ps_q[:], ones_t[:].bitcast(F32R), sq[:].bitcast(F32R), start=(d == 0), stop=(d == 2))
            # stats: M = s/384; var = q/384 - M^2; R = exp(-0.5*ln(var+eps))
            Mt = sp.tile([128, CH], F32, tag="Mt")
            Rt = sp.tile([128, CH], F32, tag="Rt")
            tmp = sp.tile([128, CH], F32, tag="tmp")
            nc.scalar.activation(out=Mt[:], in_=ps_s[:], func=AF.Copy, scale=1.0 / D)
            nc.vector.tensor_mul(tmp[:], Mt[:], Mt[:])
            nc.vector.scalar_tensor_tensor(out=tmp[:], in0=ps_q[:], scalar=1.0 / D, in1=tmp[:], op0=OP.mult, op1=OP.subtract)
            nc.scalar.activation(out=tmp[:], in_=tmp[:], func=AF.Ln, bias=eps_t[:])
            nc.scalar.activation(out=Rt[:], in_=tmp[:], func=AF.Exp, scale=-0.5)
            # normalize in place: zn = (z - M) * R  (M/R broadcast across dtiles)
            Mb = bass.AP(tensor=Mt[:].tensor, offset=Mt[:].offset, ap=[Mt[:].ap[0], [0, 3], [1, CH]])
            Rb = bass.AP(tensor=Rt[:].tensor, offset=Rt[:].offset, ap=[Rt[:].ap[0], [0, 3], [1, CH]])
            z3 = zt[:].rearrange("p (d n) -> p d n", d=3)
            nc.vector.tensor_sub(z3.bitcast(F32R), z3, Mb)
            nc.vector.tensor_mul(z3.bitcast(F32R), z3, Rb)
            # q row
            ps_qq = pp.tile([128, CH], F32, tag="ps")
            for k in range(3):
                nc.tensor.matmul(ps_qq[0:1, :], wts["qkv"][k][:, 0:1].bitcast(F32R), zt[:, k * CH : (k + 1) * CH].bitcast(F32R), start=(k == 0), stop=(k == 2))
            # softmax numerator: cs = exp(q + qB) per patch; Z partial via accum
            cs = sp.tile([1, CH], F32, tag="cs")
            qv = patch_view(ps_qq[0:1, :])
            cv_ = patch_view(cs[:])
            for ph in range(2):
                for pw_ in range(2):
                    nc.scalar.activation(
                        out=cv_[:, :, ph, :, pw_].bitcast(F32R),
                        in_=qv[:, :, ph, :, pw_],
                        func=AF.Exp,
                        bias=wts["qB"][0:1, 0:1],
                        accum_out=a["Zp"][0:1, 2 * ph + pw_, chunk : chunk + 1],
                    )
            # broadcast cs across 128 partitions via a K=1 ones matmul
            ps_cb = pp.tile([128, CH], F32, tag="ps")
            nc.tensor.matmul(ps_cb[:], ones_t[0:1, :].bitcast(F32R), cs[:].bitcast(F32R), start=True, stop=True)
            csb = sp.tile([128, CH], F32, tag="csb")
            nc.scalar.copy(csb[:], ps_cb[:])
            # s += zn * cs  (token-weighted z sum, accumulated via identity MM)
            for d in range(3):
                junk = sp.tile([128, CH], F32, tag="junk", name="junk")
                nc.vector.tensor_mul(junk[:].bitcast(F32R), zt[:, d * CH : (d + 1) * CH], csb[:])
                nc.tensor.matmul(sacc[d][:], ident_t[:].bitcast(F32R), junk[:].bitcast(F32R), start=(chunk == 0), stop=(chunk == NCH - 1))
            for m in range(3):
                ps_v = pp.tile([128, CH], F32, tag="ps")
                for k in range(3):
                    nc.tensor.matmul(ps_v[:], wts["qkv"][k][:, 1 + D + m * 128 : 1 + D + (m + 1) * 128].bitcast(F32R), zt[:, k * CH : (k + 1) * CH].bitcast(F32R), start=(k == 0), stop=(k == 2))
                vt = sp.tile([128, CH], F32, tag="vt", name="vt")
                nc.scalar.activation(out=vt[:], in_=ps_v[:], func=AF.Relu, bias=wts["vB"][:, m : m + 1])
                nc.sync.dma_start(out=vdst[m * 128 : (m + 1) * 128, chunk * CH : (chunk + 1) * CH], in_=vt[:])

        def finalize_cv(bi, sacc):
            """cv = Wk^T (patch-reduced s) / Z + kB."""
            a = att[bi]
            wts = bw[bi]
            # patch-reduce pinned psum s [128, 512] -> [128, 3, 4]
            sred = sp.tile([128, 3, 4], F32, tag="sred")
            for d in range(3):
                pv = patch_view(sacc[d][:])
                for ph in range(2):
                    for pw_ in range(2):
                        with nc.allow_low_precision(reason="f32r is bit-identical fp32; tag for matmul rhs"):
                            nc.vector.tensor_reduce(
                                sred[:, d, 2 * ph + pw_ : 2 * ph + pw_ + 1].bitcast(F32R),
                                pv[:, :, ph, :, pw_],
                                axis=mybir.AxisListType.XY,
                                op=OP.add,
                            )
            # cv_raw[m] = sum_k Wk[k][:, m*128:(m+1)*128]^T @ sred[k]
            ps_cv = pp.tile([128, 3, 4], F32, tag="ps")
            for m in range(3):
                for k in range(3):
                    nc.tensor.matmul(ps_cv[:, m, :], wts["qkv"][k][:, 1 + m * 128 : 1 + (m + 1) * 128].bitcast(F32R), sred[:, k, :].bitcast(F32R), start=(k == 0), stop=(k == 2))
            # 1/Z per patch, broadcast via DRAM bounce
            zsum = sp.tile([1, 4], F32, tag="zsum")
            zs = sp.tile([1, 4], F32, tag="zs")
            nc.vector.tensor_reduce(zsum[:], a["Zp"][:], axis=mybir.AxisListType.X, op=OP.add)
            nc.vector.reciprocal(zs[:], zsum[:])
            nc.sync.dma_start(out=zsd[bi : bi + 1, :], in_=zs[:])
            zb = sp.tile([128, 4], F32, tag="zb")
            zrow = zsd[bi, :]
            nc.sync.dma_start(out=zb[:], in_=bass.AP(tensor=zrow.tensor, offset=zrow.offset, ap=[[0, 128], [1, 4]]))
            # cvf = cv_raw * (1/Z) + kB
            for d in range(3):
                nc.vector.tensor_mul(a["cvf"][:, d, :], ps_cv[:, d, :], zb[:])
                nc.vector.tensor_scalar_add(out=a["cvf"][:, d, :], in0=a["cvf"][:, d, :], scalar1=wts["kB"][:, d : d + 1])

        def pass_b(bi, zt, vt, chunk, zdst, wp):
            """Pass B for block bi: returns new-z SBUF tile [128, 3*CH].

            zt: residual z chunk (f32), vt: relu'd v chunk (f32). Applies
            v*cv -> wo -> +z, then LN2/FFN residual. Writes result to zdst.
            """
            a = att[bi]
            wts = bw[bi]
            # v scaled by cv: per (dtile, patch) per-partition scalar (DVE)
            for d in range(3):
                vv = patch_view(vt[:, d * CH : (d + 1) * CH])
                for ph in range(2):
                    for pw_ in range(2):
                        sub = vv[:, :, ph, :, pw_]
                        nc.vector.tensor_scalar_mul(out=sub.bitcast(F32R), in0=sub, scalar1=a["cvf"][:, d, 2 * ph + pw_ : 2 * ph + pw_ + 1])
            # wo matmul + residual
            za = wp.tile([128, 3 * CH], F32, tag="za", name="za")
            for m in range(3):
                ps_o = pp.tile([128, CH], F32, tag="ps")
                for k in range(3):
                    nc.tensor.matmul(ps_o[:], wts["wo"][k][:, m * 128 : (m + 1) * 128].bitcast(F32R), vt[:, k * CH : (k + 1) * CH].bitcast(F32R), start=(k == 0), stop=(k == 2))
                nc.vector.scalar_tensor_tensor(out=za[:, m * CH : (m + 1) * CH].bitcast(F32R), in0=ps_o[:], scalar=wts["woB"][:, m : m + 1], in1=zt[:, m * CH : (m + 1) * CH], op0=OP.add, op1=OP.add)
            # LN2 stats on za
            ps_s = pp.tile([128, CH], F32, tag="ps")
            ps_q = pp.tile([128, CH], F32, tag="ps")
            for d in range(3):
                sq = sp.tile([128, CH], F32, tag="sq", name="sq")
                nc.scalar.activation(out=sq[:].bitcast(F32R), in_=za[:, d * CH : (d + 1) * CH], func=AF.Square)
                nc.tensor.matmul(ps_s[:], ones_t[:].bitcast(F32R), za[:, d * CH : (d + 1) * CH].bitcast(F32R), start=(d == 0), stop=(d == 2))
                nc.tensor.matmul(ps_q[:], ones_t[:].bitcast(F32R), sq[:].bitcast(F32R), start=(d == 0), stop=(d == 2))
            Mt = sp.tile([128, CH], F32, tag="Mt")
            Rt = sp.tile([128, CH], F32, tag="Rt")
            tmp = sp.tile([128, CH], F32, tag="tmp")
            nc.scalar.activation(out=Mt[:], in_=ps_s[:], func=AF.Copy, scale=1.0 / D)
            nc.vector.tensor_mul(tmp[:], Mt[:], Mt[:])
            nc.vector.scalar_tensor_tensor(out=tmp[:], in0=ps_q[:], scalar=1.0 / D, in1=tmp[:], op0=OP.mult, op1=OP.subtract)
            nc.scalar.activation(out=tmp[:], in_=tmp[:], func=AF.Ln, bias=eps_t[:])
            nc.scalar.activation(out=Rt[:], in_=tmp[:], func=AF.Exp, scale=-0.5)
            zn = wp.tile([128, 3 * CH], F32, tag="zn", name="zn")
            Mb = bass.AP(tensor=Mt[:].tensor, offset=Mt[:].offset, ap=[Mt[:].ap[0], [0, 3], [1, CH]])
            Rb = bass.AP(tensor=Rt[:].tensor, offset=Rt[:].offset, ap=[Rt[:].ap[0], [0, 3], [1, CH]])
            z3 = za[:].rearrange("p (d n) -> p d n", d=3)
            zn3 = zn[:].rearrange("p (d n) -> p d n", d=3)
            nc.vector.tensor_sub(zn3.bitcast(F32R), z3, Mb)
            nc.vector.tensor_mul(zn3.bitcast(F32R), zn3, Rb)
            # FFN: ff1 (6 m-tiles, silu) -> ht SBUF, then ff2 m-tiles -> + za
            ht = wp.tile([128, 6 * CH], F32, tag="ht", name="ht", bufs=1)
            for m in range(6):
                ps_1 = pp.tile([128, CH], F32, tag="ps")
                for k in range(3):
                    nc.tensor.matmul(ps_1[:], wts["ff1"][k][:, m * 128 : (m + 1) * 128].bitcast(F32R), zn[:, k * CH : (k + 1) * CH].bitcast(F32R), start=(k == 0), stop=(k == 2))
                nc.scalar.activation(out=ht[:, m * CH : (m + 1) * CH].bitcast(F32R), in_=ps_1[:], func=(AF.Square if SIM_SAFE else AF.Silu), bias=wts["ff1B"][:, m : m + 1])
            zb_ = sp.tile([128, 3 * CH], F32, tag="zt", name="zb_", bufs=3)
            for m in range(3):
                ps_f = pp.tile([128, CH], F32, tag="ps")
                for k in range(6):
                    nc.tensor.matmul(ps_f[:], wts["ff2"][k][:, m * 128 : (m + 1) * 128].bitcast(F32R), ht[:, k * CH : (k + 1) * CH].bitcast(F32R), start=(k == 0), stop=(k == 5))
                nc.vector.scalar_tensor_tensor(out=zb_[:, m * CH : (m + 1) * CH].bitcast(F32R), in0=ps_f[:], scalar=wts["ff2B"][:, m : m + 1], in1=za[:, m * CH : (m + 1) * CH], op0=OP.add, op1=OP.add)
                if zdst is not None:
                    nc.sync.dma_start(out=zdst[m * 128 : (m + 1) * 128, chunk * CH : (chunk + 1) * CH], in_=zb_[:, m * CH : (m + 1) * CH])
            return zb_

        # ================= PHASE 1: conv + pw1 + block0 pass A =================
        p1_cm = tc.tile_pool(name="p1", bufs=2)
        p1 = p1_cm.__enter__()
        sacc0_cm = tc.tile_pool(name="sacc0", bufs=1, space="PSUM")
        sacc0p = sacc0_cm.__enter__()
        sacc0 = [sacc0p.tile([128, CH], F32, tag=f"sacc0_{d}", name=f"sacc0_{d}") for d in range(3)]
        xslices = {}

        xbufs = [p1.tile([128, 2, PSL], F32, tag=f"xps{b}", name=f"xps{b}", bufs=1) for b in range(4)]

        def load_slice(ts_):
            xs = xbufs[ts_ % 4]
            if ts_ < 4:
                # zero the pad border once per buffer (borders never overwritten)
                nc.sync.dma_start(out=xs[:].bitcast(F32R), in_=zpad[:, :].bitcast(F32R))
            for cti in range(2):
                dst = xs[:, cti, :].rearrange("p (h w) -> p h w", h=PW)
                nc.sync.dma_start(out=dst[:, 1:33, 1:33].bitcast(F32R), in_=x_in[cti * 128 : (cti + 1) * 128, ts_, :, :].bitcast(F32R))
            xslices[ts_] = xs

        for t in range(T):
            for ts_ in (t - 1, t, t + 1):
                if 0 <= ts_ < T and ts_ not in xslices:
                    load_slice(ts_)
            yact = p1.tile([128, 2, H * W], F32, tag="yact", name="yact")
            for half in range(2):
                for cti in range(2):
                    # dwconv via 27 diagonal matmuls into PSUM (padded slices,
                    # full even-width windows satisfy fp32r MM ISA rules)
                    acc = pp.tile([128, CH], F32, tag="ps")
                    taps = []
                    for dt in range(3):
                        ts_ = t + dt - 1
                        if not (0 <= ts_ < T):
                            continue
                        for dh in range(3):
                            for dw in range(3):
                                taps.append((ts_, dt * 9 + dh * 3 + dw, dh, dw))
                    for ti, (ts_, tap, dh, dw) in enumerate(taps):
                        xv = xslices[ts_][:, cti, :].rearrange("p (h w) -> p h w", h=PW)
                        rhs = xv[:, half * 16 + dh : half * 16 + dh + 16, dw : dw + 32]
                        nc.tensor.matmul(
                            acc[:],
                            diag_t[cti * 27 + tap][:].bitcast(F32R),
                            rhs.bitcast(F32R),
                            start=(ti == 0),
                            stop=(ti == len(taps) - 1),
                        )
                    nc.scalar.activation(out=yact[:, cti, half * CH : (half + 1) * CH].bitcast(F32R), in_=acc[:], func=(AF.Square if SIM_SAFE else AF.Silu), bias=dwb_t[:, cti : cti + 1])
                chunk = 2 * t + half
                zt = sp.tile([128, 3 * CH], F32, tag="zt", name="zt", bufs=3)
                for m in range(3):
                    ps1 = pp.tile([128, CH], F32, tag="ps")
                    for k in range(2):
                        nc.tensor.matmul(ps1[:], pw1_t[k][:, m * 128 : (m + 1) * 128].bitcast(F32R), yact[:, k, half * CH : (half + 1) * CH].bitcast(F32R), start=(k == 0), stop=(k == 1))
                    nc.scalar.activation(out=zt[:, m * CH : (m + 1) * CH].bitcast(F32R), in_=ps1[:], func=AF.Identity, bias=pw1b_t[:, m : m + 1])
                    nc.sync.dma_start(out=z0[m * 128 : (m + 1) * 128, chunk * CH : (chunk + 1) * CH], in_=zt[:, m * CH : (m + 1) * CH])
                if STAGE >= 2:
                    ln_then_qkv_attn(0, zt, chunk, sacc0)
        p1_cm.__exit__(None, None, None)
        p23 = ctx.enter_context(tc.tile_pool(name="p23", bufs=2))
        NO_P2 = os.environ.get("KERNEL_NO_P2", "0") == "1"
        if STAGE >= 2:
            finalize_cv(0, sacc0)
            sacc0_cm.__exit__(None, None, None)
            sacc1_cm = tc.tile_pool(name="sacc1", bufs=1, space="PSUM")
            sacc1p = sacc1_cm.__enter__()
            sacc1 = [sacc1p.tile([128, CH], F32, tag=f"sacc1_{d}", name=f"sacc1_{d}") for d in range(3)]

            # ============= PHASE 2: block0 pass B + block1 pass A =============
            for chunk in range(NCH if not NO_P2 else 0):
                zt = sp.tile([128, 3 * CH], F32, tag="zt", name="zt", bufs=3)
                vt = p23.tile([128, 3 * CH], F32, tag="vt2", name="vt2")
                for m in range(3):
                    nc.sync.dma_start(out=zt[:, m * CH : (m + 1) * CH], in_=z0[m * 128 : (m + 1) * 128, chunk * CH : (chunk + 1) * CH])
                    nc.sync.dma_start(out=vt[:, m * CH : (m + 1) * CH].bitcast(F32R), in_=v0[m * 128 : (m + 1) * 128, chunk * CH : (chunk + 1) * CH].bitcast(F32R))
                zb_ = pass_b(0, zt, vt, chunk, z1, p23)
                if STAGE >= 3:
                    ln_then_qkv_attn(1, zb_, chunk, sacc1)
        if STAGE >= 3:
            finalize_cv(1, sacc1)
            sacc1_cm.__exit__(None, None, None)

            # ================= PHASE 3: block1 pass B + pw2 =================
            for chunk in range(NCH):
                zt = sp.tile([128, 3 * CH], F32, tag="zt", name="zt", bufs=3)
                vt = p23.tile([128, 3 * CH], F32, tag="vt2", name="vt2")
                for m in range(3):
                    nc.sync.dma_start(out=zt[:, m * CH : (m + 1) * CH], in_=z1[m * 128 : (m + 1) * 128, chunk * CH : (chunk + 1) * CH])
                    nc.sync.dma_start(out=vt[:, m * CH : (m + 1) * CH].bitcast(F32R), in_=v1[m * 128 : (m + 1) * 128, chunk * CH : (chunk + 1) * CH].bitcast(F32R))
                zb_ = pass_b(1, zt, vt, chunk, None, p23)
                ot = p23.tile([128, 2 * CH], F32, tag="ot", name="ot", bufs=1)
                for m in range(2):
                    ps2 = pp.tile([128, CH], F32, tag="ps")
                    for k in range(3):
                        nc.tensor.matmul(ps2[:], pw2_t[k][:, m * 128 : (m + 1) * 128].bitcast(F32R), zb_[:, k * CH : (k + 1) * CH].bitcast(F32R), start=(k == 0), stop=(k == 2))
                    nc.scalar.activation(out=ot[:, m * CH : (m + 1) * CH], in_=ps2[:], func=AF.Identity, bias=pw2b_t[:, m : m + 1])
                    nc.sync.dma_start(out=out[m * 128 : (m + 1) * 128, chunk * CH : (chunk + 1) * CH], in_=ot[:, m * CH : (m + 1) * CH])

    nc.compile()
    return nc


_NC = None


def _get_nc():
    global _NC
    if _NC is None:
        _NC = build()
    return _NC


def kernel(**inputs):
    inputs = {k: np.asarray(v, dtype=np.float32) for k, v in inputs.items()}
    x = inputs["x"]
    dw = inputs["dw_w"].reshape(C, 27)
    diag = np.zeros(((NDIAG + 1) * 128, 128), dtype=np.float32)
    idx = np.arange(128)
    for cti in range(2):
        for tap in range(27):
            blkrows = (cti * 27 + tap) * 128
            diag[blkrows + idx, idx] = dw[cti * 128 : (cti + 1) * 128, tap]
    diag[NDIAG * 128 + idx, idx] = 1.0  # identity
    base = {
        "dwDiag": diag,
        "zpad": np.zeros((128, 2 * PSL), dtype=np.float32),
        "dwB": inputs["dw_b"].reshape(C, 1),
        "pw1W": np.ascontiguousarray(inputs["pw1_w"]),
        "pw1B": inputs["pw1_b"].reshape(D, 1),
        "pw2W": np.ascontiguousarray(inputs["pw2_w"]),
        "pw2B": inputs["pw2_b"].reshape(OUTC, 1),
    }
    for i in range(NBLK):
        qkvW = inputs["ln1_g"][i][:, None] * inputs["qkv_w"][i]
        qkvB = inputs["ln1_b"][i] @ inputs["qkv_w"][i] + inputs["qkv_b"][i]
        ff1W = inputs["ln2_g"][i][:, None] * inputs["ff1_w"][i]
        ff1B = inputs["ln2_b"][i] @ inputs["ff1_w"][i] + inputs["ff1_b"][i]
        base.update(
            {
                f"qkvW{i}": np.ascontiguousarray(qkvW),
                f"qB{i}": qkvB[0:1].reshape(1, 1),
                f"kB{i}": qkvB[1 : 1 + D].reshape(D, 1),
                f"vB{i}": qkvB[1 + D :].reshape(D, 1),
                f"woW{i}": np.ascontiguousarray(inputs["wo_w"][i]),
                f"woB{i}": inputs["wo_b"][i].reshape(D, 1),
                f"ff1W{i}": np.ascontiguousarray(ff1W),
                f"ff1B{i}": ff1B.reshape(FF, 1),
                f"ff2W{i}": np.ascontiguousarray(inputs["ff2_w"][i]),
                f"ff2B{i}": inputs["ff2_b"][i].reshape(D, 1),
            }
        )
    in_maps = [dict(base, x=np.ascontiguousarray(x[b])) for b in range(B)]
    nc = _get_nc()
    trace = bool(int(os.environ.get("KERNEL_TRACE", "0")))
    res = run_bass_kernel_spmd(nc, in_maps, list(range(B)), trace=trace)
    kernel.last_exec_ns = res.exec_time_ns
    kernel.last_profile = res.profile_json
    outs = [res.results[b]["out"].reshape(OUTC, T, H, W) for b in range(B)]
    kernel.last_results = res.results
    return np.stack(outs).astype(np.float32)


# revision 20
# speedup vs baseline: 1.6983x; 1.6983x over previous
"""MobileViTV2 block kernel for 8 TRN2 NeuronCores (data-parallel over batch).

Layout: d-major everywhere — features on SBUF partitions, tokens on the free
axis, token order n = t*1024 + h*32 + w (natural). Patch id of a token is
(h&1, w&1), recoverable from free-index bits, so attention runs on natural
order with strided sub-APs and nothing is ever transposed or scattered.

All GEMM inputs are bf16 (weights cast host-side, activations written bf16 by
the producing ACT/DVE op); PSUM accumulation stays fp32. The depthwise conv
runs on the PE as 27 diagonal matmuls per output tile (diag matrices built
host-side). The k GEMM is algebraically removed: cv = Wk^T (sum_n cs_n*zn_n),
with the weighted token sum accumulated into pinned PSUM banks via identity
matmuls and Wk applied once per block at finalize. LN rsqrt is a single ACT
Rsqrt op. Chunks are processed in lockstep pairs and phase 1 is software-
pipelined (conv/pw1 of pair N+1 is emitted before pass A of pair N) so the
PE never drains while ACT/DVE work through the LN chain.
"""

import sys

sys.path.insert(0, "/opt/trn_rl_repo")
import os
import numpy as np
import ml_dtypes
from contextlib import ExitStack

import concourse.bass as bass
import concourse.mybir as mybir
import concourse.tile as tile
from concourse import bacc
from concourse.bass_utils import run_bass_kernel_spmd

F32 = mybir.dt.float32
BF16 = mybir.dt.bfloat16
AF = mybir.ActivationFunctionType
OP = mybir.AluOpType
BF = ml_dtypes.bfloat16

B, C, T, H, W = 8, 256, 16, 32, 32
D, OUTC, NBLK, FF = 384, 256, 2, 768
NTOK = T * H * W
CH = 512  # tokens per chunk; chunks processed in pairs (one t-slice)
NCH = NTOK // CH  # 32
PW = 34  # padded spatial row
PSL = PW * PW  # padded slice 1156
EPS = 1e-5
NDIAG = 2 * 27

STAGE = int(os.environ.get("KERNEL_STAGE", "3"))
SIM_SAFE = bool(int(os.environ.get("KERNEL_SIM_SAFE", "0")))


def _w_tiles(nc, wpool, name, dram, kdim, mdim):
    """Load a [K, M] DRAM bf16 weight as ceil(K/128) SBUF lhsT tiles."""
    tiles = []
    for ki in range((kdim + 127) // 128):
        kk = min(128, kdim - ki * 128)
        t = wpool.tile([128, mdim], BF16, tag=f"{name}{ki}")
        nc.sync.dma_start(out=t[:kk, :], in_=dram[ki * 128 : ki * 128 + kk, :])
        tiles.append(t)
    return tiles


def _bias_tile(nc, wpool, name, dram, n):
    """Load a [n,1] DRAM fp32 bias as a [128, ceil(n/128)] SBUF tile."""
    nt = (n + 127) // 128
    t = wpool.tile([128, nt], F32, tag=name)
    for ki in range(nt):
        kk = min(128, n - ki * 128)
        nc.sync.dma_start(out=t[:kk, ki : ki + 1], in_=dram[ki * 128 : ki * 128 + kk, :])
    return t


def patch_view(ap):
    """[p, 512] -> [p, 8, 2, 16, 2]; dims 2/4 are the (ph, pw) patch bits."""
    return ap.rearrange("p (a b c d) -> p a b c d", a=8, b=2, c=16, d=2)


def build():
    nc = bacc.Bacc("TRN2", target_bir_lowering=False, debug=False, num_devices=8)

    x_in = nc.dram_tensor("x", [C, T, H, W], BF16, kind="ExternalInput").ap()
    zpad = nc.dram_tensor("zpad", [128, 2 * PSL], BF16, kind="ExternalInput").ap()
    dwDiag = nc.dram_tensor("dwDiag", [(NDIAG + 1) * 128, 128], BF16, kind="ExternalInput").ap()
    dwB = nc.dram_tensor("dwB", [C, 1], F32, kind="ExternalInput").ap()
    pw1W = nc.dram_tensor("pw1W", [C, D], BF16, kind="ExternalInput").ap()
    pw1B = nc.dram_tensor("pw1B", [D, 1], F32, kind="ExternalInput").ap()
    pw2W = nc.dram_tensor("pw2W", [D, OUTC], BF16, kind="ExternalInput").ap()
    pw2B = nc.dram_tensor("pw2B", [OUTC, 1], F32, kind="ExternalInput").ap()
    blk = []
    for i in range(NBLK):
        blk.append(
            dict(
                qkvW=nc.dram_tensor(f"qkvW{i}", [D, 1 + 2 * D], BF16, kind="ExternalInput").ap(),
                qB=nc.dram_tensor(f"qB{i}", [1, 1], F32, kind="ExternalInput").ap(),
                kB=nc.dram_tensor(f"kB{i}", [D, 1], F32, kind="ExternalInput").ap(),
                vB=nc.dram_tensor(f"vB{i}", [D, 1], F32, kind="ExternalInput").ap(),
                woW=nc.dram_tensor(f"woW{i}", [D, D], BF16, kind="ExternalInput").ap(),
                woB=nc.dram_tensor(f"woB{i}", [D, 1], F32, kind="ExternalInput").ap(),
                ff1W=nc.dram_tensor(f"ff1W{i}", [D, FF], BF16, kind="ExternalInput").ap(),
                ff1B=nc.dram_tensor(f"ff1B{i}", [FF, 1], F32, kind="ExternalInput").ap(),
                ff2W=nc.dram_tensor(f"ff2W{i}", [FF, D], BF16, kind="ExternalInput").ap(),
                ff2B=nc.dram_tensor(f"ff2B{i}", [D, 1], F32, kind="ExternalInput").ap(),
            )
        )
    out = nc.dram_tensor("out", [OUTC, NTOK], F32, kind="ExternalOutput").ap()
    z0 = nc.dram_tensor("z0", [D, NTOK], BF16, kind="ExternalOutput").ap()
    z1 = nc.dram_tensor("z1", [D, NTOK], BF16, kind="ExternalOutput").ap()
    v0 = nc.dram_tensor("v0", [D, NTOK], BF16).ap()
    v1 = nc.dram_tensor("v1", [D, NTOK], BF16).ap()
    zsd = nc.dram_tensor("zsd", [NBLK, 4], F32).ap()

    with ExitStack() as ctx:
        tc = ctx.enter_context(tile.TileContext(nc))
        wpool = ctx.enter_context(tc.tile_pool(name="w", bufs=1))
        sp = ctx.enter_context(tc.tile_pool(name="s", bufs=4))
        pp = ctx.enter_context(tc.tile_pool(name="ps", bufs=5, space="PSUM"))
        cvp = ctx.enter_context(tc.tile_pool(name="cv", bufs=1))

        # ---- weights ----
        diag_t = _w_tiles(nc, wpool, "dwDiag", dwDiag, (NDIAG + 1) * 128, 128)
        ident_t = diag_t[NDIAG]
        dwb_t = _bias_tile(nc, wpool, "dwB", dwB, C)
        pw1_t = _w_tiles(nc, wpool, "pw1W", pw1W, C, D)
        pw1b_t = _bias_tile(nc, wpool, "pw1B", pw1B, D)
        pw2_t = _w_tiles(nc, wpool, "pw2W", pw2W, D, OUTC)
        pw2b_t = _bias_tile(nc, wpool, "pw2B", pw2B, OUTC)
        bw = []
        for i in range(NBLK):
            bw.append(
                dict(
                    qkv=_w_tiles(nc, wpool, f"qkvW{i}_", blk[i]["qkvW"], D, 1 + 2 * D),
                    qB=_bias_tile(nc, wpool, f"qB{i}", blk[i]["qB"], 1),
                    kB=_bias_tile(nc, wpool, f"kB{i}", blk[i]["kB"], D),
                    vB=_bias_tile(nc, wpool, f"vB{i}", blk[i]["vB"], D),
                    wo=_w_tiles(nc, wpool, f"woW{i}_", blk[i]["woW"], D, D),
                    woB=_bias_tile(nc, wpool, f"woB{i}", blk[i]["woB"], D),
                    ff1=_w_tiles(nc, wpool, f"ff1W{i}_", blk[i]["ff1W"], D, FF),
                    ff1B=_bias_tile(nc, wpool, f"ff1B{i}", blk[i]["ff1B"], FF),
                    ff2=_w_tiles(nc, wpool, f"ff2W{i}_", blk[i]["ff2W"], FF, D),
                    ff2B=_bias_tile(nc, wpool, f"ff2B{i}", blk[i]["ff2B"], D),
                )
            )
        ones_t = wpool.tile([128, 128], BF16, tag="ones")
        nc.vector.memset(ones_t[:], 1.0)
        eps_t = wpool.tile([128, 1], F32, tag="eps")
        nc.vector.memset(eps_t[:], EPS)

        att = []
        for i in range(NBLK):
            att.append(
                dict(
                    Zp=cvp.tile([1, 4, NCH], F32, tag=f"Zp{i}", name=f"Zp{i}"),
                    cvf=cvp.tile([128, 3, 4], F32, tag=f"cvf{i}", name=f"cvf{i}"),
                )
            )
            nc.vector.memset(att[i]["Zp"][:], 1.0)

        def ln_stats(zt3, tag):
            """LN stats for a [128, 3, CH] bf16 chunk -> (Mt bf16, Rt bf16)."""
            ps_s = pp.tile([128, CH], F32, tag="ps")
            ps_q = pp.tile([128, CH], F32, tag="ps")
            for d in range(3):
                sq = sp.tile([128, CH], BF16, tag="sq", name=f"sq{tag}")
                nc.scalar.activation(out=sq[:], in_=zt3[:, d, :], func=AF.Square)
                nc.tensor.matmul(ps_s[:], ones_t[:], zt3[:, d, :], start=(d == 0), stop=(d == 2))
                nc.tensor.matmul(ps_q[:], ones_t[:], sq[:], start=(d == 0), stop=(d == 2))
            Mt = sp.tile([128, CH], BF16, tag="Mt", name=f"Mt{tag}")
            Rt = sp.tile([128, CH], BF16, tag="Rt", name=f"Rt{tag}")
            tmp = sp.tile([128, CH], F32, tag="tmp", name=f"tmp{tag}")
            nc.scalar.activation(out=Mt[:], in_=ps_s[:], func=AF.Copy, scale=1.0 / D)
            nc.vector.tensor_mul(tmp[:], Mt[:], Mt[:])
            nc.vector.scalar_tensor_tensor(out=tmp[:], in0=ps_q[:], scalar=1.0 / D, in1=tmp[:], op0=OP.mult, op1=OP.subtract)
            nc.scalar.activation(out=tmp[:], in_=tmp[:], func=AF.Ln, bias=eps_t[:])
            nc.scalar.activation(out=Rt[:], in_=tmp[:], func=AF.Exp, scale=-0.5)
            return Mt, Rt

        def ln_apply(src3, dst3, Mt, Rt):
            """dst = (src - M) * R, all bf16, M/R broadcast across dtiles."""
            Mb = bass.AP(tensor=Mt[:].tensor, offset=Mt[:].offset, ap=[Mt[:].ap[0], [0, 3], [1, CH]])
            Rb = bass.AP(tensor=Rt[:].tensor, offset=Rt[:].offset, ap=[Rt[:].ap[0], [0, 3], [1, CH]])
            nc.vector.tensor_sub(dst3, src3, Mb)
            nc.vector.tensor_mul(dst3, dst3, Rb)

        def pass_a(bi, items, sacc):
            """Pass A for block bi on chunk items [(chunk, zt)]; zt [128,3*CH] bf16.

            Stage-interleaved across the pair so same-ACT-set ops batch and the
            PE always has the sibling chunk's matmuls queued.
            """
            a = att[bi]
            wts = bw[bi]
            vdst = v0 if bi == 0 else v1
            st = [dict(chunk=c, zt=zt, z3=zt[:].rearrange("p (d n) -> p d n", d=3)) for c, zt in items]
            for s in st:
                s["Mt"], s["Rt"] = ln_stats(s["z3"], f"a{s['chunk'] & 1}")
            for s in st:
                ln_apply(s["z3"], s["z3"], s["Mt"], s["Rt"])
            for s in st:
                ps_qq = pp.tile([128, CH], F32, tag="ps")
                for k in range(3):
                    nc.tensor.matmul(ps_qq[0:1, :], wts["qkv"][k][:, 0:1], s["z3"][:, k, :], start=(k == 0), stop=(k == 2))
                s["ps_qq"] = ps_qq
            for s in st:
                cs = sp.tile([1, CH], BF16, tag="cs", name=f"cs{s['chunk'] & 1}")
                qv = patch_view(s["ps_qq"][0:1, :])
                cv_ = patch_view(cs[:])
                for ph in range(2):
                    for pw_ in range(2):
                        nc.scalar.activation(
                            out=cv_[:, :, ph, :, pw_],
                            in_=qv[:, :, ph, :, pw_],
                            func=AF.Exp,
                            bias=wts["qB"][0:1, 0:1],
                            accum_out=a["Zp"][0:1, 2 * ph + pw_, s["chunk"] : s["chunk"] + 1],
                        )
                s["cs"] = cs
            for s in st:
                ps_cb = pp.tile([128, CH], F32, tag="ps")
                nc.tensor.matmul(ps_cb[:], ones_t[0:1, :], s["cs"][:], start=True, stop=True)
                csb = sp.tile([128, CH], BF16, tag="csb", name=f"csb{s['chunk'] & 1}")
                nc.scalar.copy(csb[:], ps_cb[:])
                s["csb"] = csb
            for s in st:
                for d in range(3):
                    junk = sp.tile([128, CH], BF16, tag="junk", name=f"junk{s['chunk'] & 1}")
                    nc.vector.tensor_mul(junk[:], s["z3"][:, d, :], s["csb"][:])
                    nc.tensor.matmul(sacc[d][:], ident_t[:], junk[:], start=(s["chunk"] == 0), stop=(s["chunk"] == NCH - 1))
            for s in st:
                for m in range(3):
                    ps_v = pp.tile([128, CH], F32, tag="ps")
                    for k in range(3):
                        nc.tensor.matmul(ps_v[:], wts["qkv"][k][:, 1 + D + m * 128 : 1 + D + (m + 1) * 128], s["z3"][:, k, :], start=(k == 0), stop=(k == 2))
                    vt = sp.tile([128, CH], BF16, tag="vt", name=f"vt{s['chunk'] & 1}")
                    nc.scalar.activation(out=vt[:], in_=ps_v[:], func=AF.Relu, bias=wts["vB"][:, m : m + 1])
                    nc.sync.dma_start(out=vdst[m * 128 : (m + 1) * 128, s["chunk"] * CH : (s["chunk"] + 1) * CH], in_=vt[:])

        def finalize_cv(bi, sacc):
            """cv = Wk^T (patch-reduced s) / Z + kB."""
            a = att[bi]
            wts = bw[bi]
            sred = sp.tile([128, 3, 4], BF16, tag="sred")
            for d in range(3):
                pv = patch_view(sacc[d][:])
                for ph in range(2):
                    for pw_ in range(2):
                        with nc.allow_low_precision(reason="patch sums of fp32 psum; bf16 ample for cv"):
                            nc.vector.tensor_reduce(
                                sred[:, d, 2 * ph + pw_ : 2 * ph + pw_ + 1],
                                pv[:, :, ph, :, pw_],
                                axis=mybir.AxisListType.XY,
                                op=OP.add,
                            )
            ps_cv = pp.tile([128, 3, 4], F32, tag="ps")
            for m in range(3):
                for k in range(3):
                    nc.tensor.matmul(ps_cv[:, m, :], wts["qkv"][k][:, 1 + m * 128 : 1 + (m + 1) * 128], sred[:, k, :], start=(m == 0 and k == 0), stop=(m == 2 and k == 2))
            zsum = sp.tile([1, 4], F32, tag="zsum")
            zs = sp.tile([1, 4], F32, tag="zs")
            nc.vector.tensor_reduce(zsum[:], a["Zp"][:], axis=mybir.AxisListType.X, op=OP.add)
            nc.vector.reciprocal(zs[:], zsum[:])
            nc.sync.dma_start(out=zsd[bi : bi + 1, :], in_=zs[:])
            zb = sp.tile([128, 4], F32, tag="zb")
            zrow = zsd[bi, :]
            nc.sync.dma_start(out=zb[:], in_=bass.AP(tensor=zrow.tensor, offset=zrow.offset, ap=[[0, 128], [1, 4]]))
            for d in range(3):
                nc.vector.tensor_mul(a["cvf"][:, d, :], ps_cv[:, d, :], zb[:])
                nc.vector.tensor_scalar_add(out=a["cvf"][:, d, :], in0=a["cvf"][:, d, :], scalar1=wts["kB"][:, d : d + 1])

        def pass_b(bi, items, zdst, wp):
            """Pass B for block bi on items [(chunk, zt, vt)] (bf16 tiles).

            Returns [(chunk, zb_)] with the new-z [128, 3*CH] bf16 tiles.
            """
            a = att[bi]
            wts = bw[bi]
            st = [dict(chunk=c, zt=zt, vt=vt) for c, zt, vt in items]
            for s in st:
                for d in range(3):
                    vv = patch_view(s["vt"][:, d * CH : (d + 1) * CH])
                    for ph in range(2):
                        for pw_ in range(2):
                            sub = vv[:, :, ph, :, pw_]
                            nc.vector.tensor_scalar_mul(out=sub, in0=sub, scalar1=a["cvf"][:, d, 2 * ph + pw_ : 2 * ph + pw_ + 1])
            for s in st:
                za = wp.tile([128, 3 * CH], BF16, tag="za", name=f"za{s['chunk'] & 1}")
                for m in range(3):
                    ps_o = pp.tile([128, CH], F32, tag="ps")
                    for k in range(3):
                        nc.tensor.matmul(ps_o[:], wts["wo"][k][:, m * 128 : (m + 1) * 128], s["vt"][:, k * CH : (k + 1) * CH], start=(k == 0), stop=(k == 2))
                    nc.vector.scalar_tensor_tensor(out=za[:, m * CH : (m + 1) * CH], in0=ps_o[:], scalar=wts["woB"][:, m : m + 1], in1=s["zt"][:, m * CH : (m + 1) * CH], op0=OP.add, op1=OP.add)
                s["za"] = za
                s["za3"] = za[:].rearrange("p (d n) -> p d n", d=3)
            for s in st:
                s["Mt"], s["Rt"] = ln_stats(s["za3"], f"b{s['chunk'] & 1}")
            for s in st:
                zn = wp.tile([128, 3 * CH], BF16, tag="zn", name=f"zn{s['chunk'] & 1}")
                ln_apply(s["za3"], zn[:].rearrange("p (d n) -> p d n", d=3), s["Mt"], s["Rt"])
                s["zn"] = zn
            for s in st:
                ht = wp.tile([128, 6 * CH], BF16, tag="ht", name=f"ht{s['chunk'] & 1}", bufs=2)
                for m in range(6):
                    ps_1 = pp.tile([128, CH], F32, tag="ps")
                    for k in range(3):
                        nc.tensor.matmul(ps_1[:], wts["ff1"][k][:, m * 128 : (m + 1) * 128], s["zn"][:, k * CH : (k + 1) * CH], start=(k == 0), stop=(k == 2))
                    nc.scalar.activation(out=ht[:, m * CH : (m + 1) * CH], in_=ps_1[:], func=(AF.Square if SIM_SAFE else AF.Silu), bias=wts["ff1B"][:, m : m + 1])
                s["ht"] = ht
            outs = []
            for s in st:
                zb_ = sp.tile([128, 3 * CH], BF16, tag="zt", name=f"zb_{s['chunk'] & 1}", bufs=6)
                for m in range(3):
                    ps_f = pp.tile([128, CH], F32, tag="ps")
                    for k in range(6):
                        nc.tensor.matmul(ps_f[:], wts["ff2"][k][:, m * 128 : (m + 1) * 128], s["ht"][:, k * CH : (k + 1) * CH], start=(k == 0), stop=(k == 5))
                    nc.vector.scalar_tensor_tensor(out=zb_[:, m * CH : (m + 1) * CH], in0=ps_f[:], scalar=wts["ff2B"][:, m : m + 1], in1=s["za"][:, m * CH : (m + 1) * CH], op0=OP.add, op1=OP.add)
                    if zdst is not None:
                        nc.sync.dma_start(out=zdst[m * 128 : (m + 1) * 128, s["chunk"] * CH : (s["chunk"] + 1) * CH], in_=zb_[:, m * CH : (m + 1) * CH])
                outs.append((s["chunk"], zb_))
            return outs

        # ================= PHASE 1: conv + pw1 + block0 pass A =================
        p1_cm = tc.tile_pool(name="p1", bufs=2)
        p1 = p1_cm.__enter__()
        sacc0_cm = tc.tile_pool(name="sacc0", bufs=1, space="PSUM")
        sacc0p = sacc0_cm.__enter__()
        sacc0 = [sacc0p.tile([128, CH], F32, tag=f"sacc0_{d}", name=f"sacc0_{d}") for d in range(3)]
        xslices = {}
        xbufs = [p1.tile([128, 2, PSL], BF16, tag=f"xps{b}", name=f"xps{b}", bufs=1) for b in range(4)]

        def load_slice(ts_):
            xs = xbufs[ts_ % 4]
            if ts_ < 4:
                nc.sync.dma_start(out=xs[:], in_=zpad[:, :])
            for cti in range(2):
                dst = xs[:, cti, :].rearrange("p (h w) -> p h w", h=PW)
                nc.sync.dma_start(out=dst[:, 1:33, 1:33], in_=x_in[cti * 128 : (cti + 1) * 128, ts_, :, :])
            xslices[ts_] = xs

        pend = []
        for t in range(T):
            for ts_ in (t - 1, t, t + 1):
                if 0 <= ts_ < T and ts_ not in xslices:
                    load_slice(ts_)
            yact = p1.tile([128, 2, H * W], BF16, tag="yact", name="yact")
            cur = []
            for half in range(2):
                for cti in range(2):
                    acc = pp.tile([128, CH], F32, tag="ps")
                    taps = []
                    for dt in range(3):
                        ts_ = t + dt - 1
                        if not (0 <= ts_ < T):
                            continue
                        for dh in range(3):
                            for dw in range(3):
                                taps.append((ts_, dt * 9 + dh * 3 + dw, dh, dw))
                    for ti, (ts_, tap, dh, dw) in enumerate(taps):
                        xv = xslices[ts_][:, cti, :].rearrange("p (h w) -> p h w", h=PW)
                        rhs = xv[:, half * 16 + dh : half * 16 + dh + 16, dw : dw + 32]
                        nc.tensor.matmul(acc[:], diag_t[cti * 27 + tap][:], rhs, start=(ti == 0), stop=(ti == len(taps) - 1))
                    nc.scalar.activation(out=yact[:, cti, half * CH : (half + 1) * CH], in_=acc[:], func=(AF.Square if SIM_SAFE else AF.Silu), bias=dwb_t[:, cti : cti + 1])
                chunk = 2 * t + half
                zt = sp.tile([128, 3 * CH], BF16, tag="zt", name=f"zt{chunk & 1}", bufs=6)
                for m in range(3):
                    ps1 = pp.tile([128, CH], F32, tag="ps")
                    for k in range(2):
                        nc.tensor.matmul(ps1[:], pw1_t[k][:, m * 128 : (m + 1) * 128], yact[:, k, half * CH : (half + 1) * CH], start=(k == 0), stop=(k == 1))
                    nc.scalar.activation(out=zt[:, m * CH : (m + 1) * CH], in_=ps1[:], func=AF.Identity, bias=pw1b_t[:, m : m + 1])
                    nc.sync.dma_start(out=z0[m * 128 : (m + 1) * 128, chunk * CH : (chunk + 1) * CH], in_=zt[:, m * CH : (m + 1) * CH])
                cur.append((chunk, zt))
            # software pipeline: run pass A for the PREVIOUS t-slice pair while
            # this slice's conv/pw1 matmuls keep the PE busy
            if STAGE >= 2 and pend:
                pass_a(0, pend, sacc0)
            pend = cur
        if STAGE >= 2 and pend:
            pass_a(0, pend, sacc0)
        p1_cm.__exit__(None, None, None)
        p23 = ctx.enter_context(tc.tile_pool(name="p23", bufs=3))
        if STAGE >= 2:
            finalize_cv(0, sacc0)
            sacc0_cm.__exit__(None, None, None)
            sacc1_cm = tc.tile_pool(name="sacc1", bufs=1, space="PSUM")
            sacc1p = sacc1_cm.__enter__()
            sacc1 = [sacc1p.tile([128, CH], F32, tag=f"sacc1_{d}", name=f"sacc1_{d}") for d in range(3)]

            # ============= PHASE 2: block0 pass B + block1 pass A =============
            for base in range(0, NCH, 2):
                items = []
                for chunk in (base, base + 1):
                    zt = sp.tile([128, 3 * CH], BF16, tag="zt", name=f"zt{chunk & 1}", bufs=6)
                    vt = p23.tile([128, 3 * CH], BF16, tag="vt2", name=f"vt2{chunk & 1}")
                    for m in range(3):
                        nc.sync.dma_start(out=zt[:, m * CH : (m + 1) * CH], in_=z0[m * 128 : (m + 1) * 128, chunk * CH : (chunk + 1) * CH])
                        nc.sync.dma_start(out=vt[:, m * CH : (m + 1) * CH], in_=v0[m * 128 : (m + 1) * 128, chunk * CH : (chunk + 1) * CH])
                    items.append((chunk, zt, vt))
                zbs = pass_b(0, items, z1, p23)
                if STAGE >= 3:
                    pass_a(1, zbs, sacc1)
        if STAGE >= 3:
            finalize_cv(1, sacc1)
            sacc1_cm.__exit__(None, None, None)

            # ================= PHASE 3: block1 pass B + pw2 =================
            for base in range(0, NCH, 2):
                items = []
                for chunk in (base, base + 1):
                    zt = sp.tile([128, 3 * CH], BF16, tag="zt", name=f"zt{chunk & 1}", bufs=6)
                    vt = p23.tile([128, 3 * CH], BF16, tag="vt2", name=f"vt2{chunk & 1}")
                    for m in range(3):
                        nc.sync.dma_start(out=zt[:, m * CH : (m + 1) * CH], in_=z1[m * 128 : (m + 1) * 128, chunk * CH : (chunk + 1) * CH])
                        nc.sync.dma_start(out=vt[:, m * CH : (m + 1) * CH], in_=v1[m * 128 : (m + 1) * 128, chunk * CH : (chunk + 1) * CH])
                    items.append((chunk, zt, vt))
                zbs = pass_b(1, items, None, p23)
                for chunk, zb_ in zbs:
                    ot = p23.tile([128, 2 * CH], F32, tag="ot", name="ot", bufs=2)
                    for m in range(2):
                        ps2 = pp.tile([128, CH], F32, tag="ps")
                        for k in range(3):
                            nc.tensor.matmul(ps2[:], pw2_t[k][:, m * 128 : (m + 1) * 128], zb_[:, k * CH : (k + 1) * CH], start=(k == 0), stop=(k == 2))
                        nc.scalar.activation(out=ot[:, m * CH : (m + 1) * CH], in_=ps2[:], func=AF.Identity, bias=pw2b_t[:, m : m + 1])
                        nc.sync.dma_start(out=out[m * 128 : (m + 1) * 128, chunk * CH : (chunk + 1) * CH], in_=ot[:, m * CH : (m + 1) * CH])

    nc.compile()
    return nc


_NC = None


def _get_nc():
    global _NC
    if _NC is None:
        _NC = build()
    return _NC


def kernel(**inputs):
    inputs = {k: np.asarray(v, dtype=np.float32) for k, v in inputs.items()}
    x = inputs["x"]
    dw = inputs["dw_w"].reshape(C, 27)
    diag = np.zeros(((NDIAG + 1) * 128, 128), dtype=np.float32)
    idx = np.arange(128)
    for cti in range(2):
        for tap in range(27):
            blkrows = (cti * 27 + tap) * 128
            diag[blkrows + idx, idx] = dw[cti * 128 : (cti + 1) * 128, tap]
    diag[NDIAG * 128 + idx, idx] = 1.0
    base = {
        "dwDiag": diag.astype(BF),
        "zpad": np.zeros((128, 2 * PSL), dtype=BF),
        "dwB": inputs["dw_b"].reshape(C, 1),
        "pw1W": np.ascontiguousarray(inputs["pw1_w"]).astype(BF),
        "pw1B": inputs["pw1_b"].reshape(D, 1),
        "pw2W": np.ascontiguousarray(inputs["pw2_w"]).astype(BF),
        "pw2B": inputs["pw2_b"].reshape(OUTC, 1),
    }
    for i in range(NBLK):
        qkvW = inputs["ln1_g"][i][:, None] * inputs["qkv_w"][i]
        qkvB = inputs["ln1_b"][i] @ inputs["qkv_w"][i] + inputs["qkv_b"][i]
        ff1W = inputs["ln2_g"][i][:, None] * inputs["ff1_w"][i]
        ff1B = inputs["ln2_b"][i] @ inputs["ff1_w"][i] + inputs["ff1_b"][i]
        base.update(
            {
                f"qkvW{i}": np.ascontiguousarray(qkvW).astype(BF),
                f"qB{i}": qkvB[0:1].reshape(1, 1),
                f"kB{i}": qkvB[1 : 1 + D].reshape(D, 1),
                f"vB{i}": qkvB[1 + D :].reshape(D, 1),
                f"woW{i}": np.ascontiguousarray(inputs["wo_w"][i]).astype(BF),
                f"woB{i}": inputs["wo_b"][i].reshape(D, 1),
                f"ff1W{i}": np.ascontiguousarray(ff1W).astype(BF),
                f"ff1B{i}": ff1B.reshape(FF, 1),
                f"ff2W{i}": np.ascontiguousarray(inputs["ff2_w"][i]).astype(BF),
                f"ff2B{i}": inputs["ff2_b"][i].reshape(D, 1),
            }
        )
    in_maps = [dict(base, x=np.ascontiguousarray(x[b]).astype(BF)) for b in range(B)]
    nc = _get_nc()
    trace = bool(int(os.environ.get("KERNEL_TRACE", "0")))
    res = run_bass_kernel_spmd(nc, in_maps, list(range(B)), trace=trace)
    kernel.last_exec_ns = res.exec_time_ns
    kernel.last_profile = res.profile_json
    outs = [res.results[b]["out"].reshape(OUTC, T, H, W) for b in range(B)]
    kernel.last_results = res.results
    return np.stack(outs).astype(np.float32)
